# revision 9
# baseline (speedup 1.0000x reference)
"""ExllamaV3 trellis-dequant linear layer on 8 Trainium2 NeuronCores.

y = x @ W,  W = diag(suh) . blockH . dequant(trellis) . blockH . diag(svh)

Sharding: column-parallel over out_features (512 cols/core). Per core:
  - trellis words are host-packed (pure layout) into a per-partition uint32
    stream V = (w[j-1]<<16)|w[j] so the on-device dequant is a uniform
    13-op DVE chain (byte extract + exact fp32-split LCG + fp16 bitcast).
  - left Hadamard + suh are folded into the weight via PE matmuls.
  - main matmul runs in fp32r (1-8-11) at bf16 streaming rate.
  - right Hadamard + svh are applied to the output tiles via PE transpose
    + matmul; the y shard is returned n-major and assembled on the host.
"""
import os
import numpy as np
from contextlib import ExitStack

import concourse.bass as bass
import concourse.tile as tile
from concourse import bacc, mybir
from concourse import bass_utils

Alu = mybir.AluOpType
f32 = mybir.dt.float32
f32r = mybir.dt.float32r
f16 = mybir.dt.float16
i32 = mybir.dt.int32
u32 = mybir.dt.uint32

TOKENS = 4096
IN_F = 4096
OUT_F = 4096
NCORES = 8
NSH = OUT_F // NCORES          # 512 out cols per core
Kt = IN_F // 16                # 256
NTS = (OUT_F // 16) // NCORES  # 32 trellis tile-cols per core
NKB = IN_F // 128              # 32 contraction chunks
NTT = TOKENS // 128            # 32 token tiles
SPP = NKB * NTS * 2 * 8        # 16384 stream elems per partition

MULT = 89226354
ADD = 64248484
C2h, C2l = MULT >> 16, MULT & 0xFFFF
_C1 = (MULT * 256) & 0xFFFFFFFF
C1h, C1l = _C1 >> 16, _C1 & 0xFFFF
ADDh, ADDl = ADD >> 16, ADD & 0xFFFF

DEQ_FD = 1024                  # dequant batch free-dim (2 k-chunks)
NBATCH = SPP // DEQ_FD         # 16

# ---------------- host-side layout helpers (pure layout, no math) ----------

_p = np.arange(128)
_a_p = _p // 16                 # kt % 8
_r_p = _p % 16                  # k % 16
_c_p = np.where(_r_p < 8, _r_p % 2, 2 + (_r_p % 2))
_tr_p = np.where(_r_p < 8, _r_p // 2, (_r_p - 8) // 2)
SH_H = (20 - 4 * _c_p).astype(np.int32)   # per-partition vh shift
SH_L = (12 - 4 * _c_p).astype(np.int32)   # per-partition vl shift

_s = np.arange(SPP)
_tc_s = _s % 8
_jh_s = (_s // 8) % 2
_nt_s = (_s // 16) % NTS
_kb_s = _s // (16 * NTS)

_KT_IDX = 8 * _kb_s[None, :] + _a_p[:, None]
_J_IDX = 8 * _tc_s[None, :] + 2 * _tr_p[:, None] + _jh_s[None, :]
_JM1_IDX = (_J_IDX - 1) % 64
_NT_IDX = np.broadcast_to(_nt_s[None, :], (128, SPP))


def _hadamard128():
    h = np.array([[1]], dtype=np.int64)
    while h.shape[0] < 128:
        h = np.block([[h, h], [h, -h]])
    return h.astype(np.float32)


def _pack_vs(w16_core):
    """[Kt, NTS, 64] uint16 (pair-swapped) -> [128, SPP] uint32 stream."""
    w0 = w16_core[_KT_IDX, _NT_IDX, _J_IDX].astype(np.uint32)
    w1 = w16_core[_KT_IDX, _NT_IDX, _JM1_IDX].astype(np.uint32)
    return (w1 << 16) | w0


# ---------------- device program ------------------------------------------

def _build_program():
    nc = bacc.Bacc(
        "TRN2",
        target_bir_lowering=False,
        debug=False,
        enable_asserts=False,
        num_devices=NCORES,
    )

    x_d = nc.dram_tensor("x_in", [TOKENS, IN_F], f32, kind="ExternalInput")
    vs_d = nc.dram_tensor("vs_in", [128, SPP], u32, kind="ExternalInput")
    suh_d = nc.dram_tensor("suh_in", [128, NKB], f32, kind="ExternalInput")
    svh_d = nc.dram_tensor("svh_in", [128, 4], f32, kind="ExternalInput")
    y_d = nc.dram_tensor("y_out", [NSH, TOKENS], f32, kind="ExternalOutput")

    had = _hadamard128()
    hL_d = nc.inline_tensor(np.ascontiguousarray(had / 128.0), name="hL")
    hR_d = nc.inline_tensor(np.ascontiguousarray(had), name="hR")
    ident_d = nc.inline_tensor(np.eye(128, dtype=np.float32), name="ident")
    shh_d = nc.inline_tensor(SH_H.reshape(128, 1), name="shh")
    shl_d = nc.inline_tensor(SH_L.reshape(128, 1), name="shl")

    with tile.TileContext(nc) as tc, ExitStack() as ctx:
        cpool = ctx.enter_context(tc.tile_pool(name="consts", bufs=1))
        hL = cpool.tile([128, 128], f32r)
        hR = cpool.tile([128, 128], f32r)
        ident = cpool.tile([128, 128], f32r)
        shh = cpool.tile([128, 1], i32)
        shl = cpool.tile([128, 1], i32)
        suh = cpool.tile([128, NKB], f32)
        svh = cpool.tile([128, 4], f32)
        nc.sync.dma_start(hL[:], hL_d.ap().bitcast(f32r))
        nc.sync.dma_start(hR[:], hR_d.ap().bitcast(f32r))
        nc.sync.dma_start(ident[:], ident_d.ap().bitcast(f32r))
        nc.sync.dma_start(shh[:], shh_d.ap())
        nc.sync.dma_start(shl[:], shl_d.ap())
        nc.sync.dma_start(suh[:], suh_d.ap())
        nc.sync.dma_start(svh[:], svh_d.ap())

        w2pool = ctx.enter_context(tc.tile_pool(name="w2", bufs=NKB))
        W2 = [w2pool.tile([128, NSH], f32r, tag="w2", name=f"w2_{i}")
              for i in range(NKB)]

        vs_pool = ctx.enter_context(tc.tile_pool(name="vsin", bufs=2))
        deq = ctx.enter_context(tc.tile_pool(name="deq", bufs=8))
        wi_pool = ctx.enter_context(tc.tile_pool(name="wi", bufs=2))
        psw_pool = ctx.enter_context(tc.tile_pool(name="psw", bufs=1, space="PSUM"))

        # ---- Phase W: dequant + left Hadamard + suh ----
        for b in range(NBATCH):
            V = vs_pool.tile([128, DEQ_FD], i32)
            nc.sync.dma_start(V[:], vs_d.ap()[:, b * DEQ_FD:(b + 1) * DEQ_FD].bitcast(i32))

            vh = deq.tile([128, DEQ_FD], i32, tag="deq")
            vl = deq.tile([128, DEQ_FD], i32, tag="deq")
            nc.vector.tensor_scalar(vh[:], V[:], shh[:], 0xFF, Alu.logical_shift_right, Alu.bitwise_and)
            nc.vector.tensor_scalar(vl[:], V[:], shl[:], 0xFF, Alu.logical_shift_right, Alu.bitwise_and)

            t1 = deq.tile([128, DEQ_FD], i32, tag="deq")
            nc.vector.tensor_scalar(t1[:], vl[:], C2l, ADDl, Alu.mult, Alu.add)
            slo = deq.tile([128, DEQ_FD], i32, tag="deq")
            nc.vector.scalar_tensor_tensor(slo[:], vh[:], C1l, t1[:], Alu.mult, Alu.add)
            t3 = deq.tile([128, DEQ_FD], i32, tag="deq")
            nc.vector.tensor_scalar(t3[:], vl[:], C2h, ADDh, Alu.mult, Alu.add)
            t4 = deq.tile([128, DEQ_FD], i32, tag="deq")
            nc.vector.scalar_tensor_tensor(t4[:], vh[:], C1h, t3[:], Alu.mult, Alu.add)
            carry = deq.tile([128, DEQ_FD], i32, tag="deq")
            nc.vector.tensor_scalar(carry[:], slo[:], 16, None, Alu.logical_shift_right)
            shi = deq.tile([128, DEQ_FD], i32, tag="deq")
            nc.vector.tensor_tensor(shi[:], t4[:], carry[:], Alu.add)
            rlo = deq.tile([128, DEQ_FD], i32, tag="deq")
            nc.vector.tensor_scalar(rlo[:], slo[:], 0x8FFF, 0x3B60, Alu.bitwise_and, Alu.bitwise_xor)
            rhi = deq.tile([128, DEQ_FD], i32, tag="deq")
            nc.vector.tensor_scalar(rhi[:], shi[:], 0x8FFF, 0x3B60, Alu.bitwise_and, Alu.bitwise_xor)

            wi = wi_pool.tile([128, DEQ_FD], f32r)
            lo16 = rlo[:].bitcast(f16).rearrange("p (f two) -> p f two", two=2)[:, :, 0]
            hi16 = rhi[:].bitcast(f16).rearrange("p (f two) -> p f two", two=2)[:, :, 0]
            nc.vector.tensor_tensor(wi[:], lo16, hi16, Alu.add)

            # left Hadamard (+1/128) per 512-col k-chunk, then suh row scale
            for q in range(DEQ_FD // NSH):
                kb = b * (DEQ_FD // NSH) + q
                psw = psw_pool.tile([128, NSH], f32)
                nc.tensor.matmul(psw[:], hL[:], wi[:, q * NSH:(q + 1) * NSH],
                                 start=True, stop=True)
                nc.scalar.mul(W2[kb][:], psw[:], suh[:, kb:kb + 1])

        # ---- Phase X + M + Y per token tile ----
        xin_pool = ctx.enter_context(tc.tile_pool(name="xin", bufs=3))
        xst_pool = ctx.enter_context(tc.tile_pool(name="xst", bufs=2))
        psx_pool = ctx.enter_context(tc.tile_pool(name="psx", bufs=2, space="PSUM"))
        psy_pool = ctx.enter_context(tc.tile_pool(name="psy", bufs=2, space="PSUM"))
        pst_pool = ctx.enter_context(tc.tile_pool(name="pst", bufs=1, space="PSUM"))
        psz_pool = ctx.enter_context(tc.tile_pool(name="psz", bufs=1, space="PSUM"))
        ysb_pool = ctx.enter_context(tc.tile_pool(name="ysb", bufs=2))
        ytT_pool = ctx.enter_context(tc.tile_pool(name="ytT", bufs=1))
        zsb_pool = ctx.enter_context(tc.tile_pool(name="zsb", bufs=2))

        ytT_group = None
        for tt in range(NTT):
            # x row-block in (two half-rows), transpose to k-major
            xst = xst_pool.tile([128, NKB * 128], f32r)
            for h in range(2):
                xin = xin_pool.tile([128, IN_F // 2], f32r, tag="xin")
                nc.sync.dma_start(
                    xin[:],
                    x_d.ap()[tt * 128:(tt + 1) * 128,
                             h * (IN_F // 2):(h + 1) * (IN_F // 2)].bitcast(f32r))
                for g in range(NKB // 8):
                    psx = psx_pool.tile([128, 512], f32r)
                    for q in range(4):
                        kb = 16 * h + 4 * g + q
                        nc.tensor.transpose(psx[:, q * 128:(q + 1) * 128],
                                            xin[:, (4 * g + q) * 128:(4 * g + q + 1) * 128],
                                            ident[:])
                    gg = 4 * h + g
                    if gg % 2 == 0:
                        nc.scalar.copy(xst[:, gg * 512:(gg + 1) * 512], psx[:])
                    else:
                        nc.vector.tensor_copy(xst[:, gg * 512:(gg + 1) * 512], psx[:])

            # main matmul: accumulate y' over all k-chunks
            psy = psy_pool.tile([128, NSH], f32)
            for kb in range(NKB):
                nc.tensor.matmul(psy[:], xst[:, kb * 128:(kb + 1) * 128],
                                 W2[kb][:],
                                 start=(kb == 0), stop=(kb == NKB - 1))

            # y side: transpose y' tiles into n-major staging
            ysb = ysb_pool.tile([128, NSH], f32r)
            nc.scalar.copy(ysb[:], psy[:])
            if tt % 4 == 0:
                ytT_group = ytT_pool.tile([128, 4 * 512], f32r, tag="ytT")
            for nb in range(4):
                pst = pst_pool.tile([128, 128], f32r)
                nc.tensor.transpose(pst[:], ysb[:, nb * 128:(nb + 1) * 128], ident[:])
                nc.scalar.copy(ytT_group[:, nb * 512 + (tt % 4) * 128:
                                         nb * 512 + (tt % 4) * 128 + 128], pst[:])

            if tt % 4 == 3:
                ttg = tt // 4
                for nb in range(4):
                    psz = psz_pool.tile([128, 512], f32)
                    nc.tensor.matmul(psz[:], hR[:],
                                     ytT_group[:, nb * 512:(nb + 1) * 512],
                                     start=True, stop=True)
                    zsb = zsb_pool.tile([128, 512], f32)
                    nc.scalar.mul(zsb[:], psz[:], svh[:, nb:nb + 1])
                    nc.sync.dma_start(
                        y_d.ap()[nb * 128:(nb + 1) * 128, ttg * 512:(ttg + 1) * 512],
                        zsb[:])

    nc.compile()
    return nc


_NC_CACHE = None
LAST_RESULT = None


def _get_program():
    global _NC_CACHE
    if _NC_CACHE is None:
        _NC_CACHE = _build_program()
    return _NC_CACHE


def kernel(x, trellis, suh, svh):
    global LAST_RESULT
    x = np.ascontiguousarray(np.asarray(x, dtype=np.float32))
    trellis = np.asarray(trellis)
    suh = np.asarray(suh, dtype=np.float32)
    svh = np.asarray(svh, dtype=np.float32)

    # host layout prep (pure re-layout of the int words)
    w16 = (trellis.astype(np.uint32) & 0xFFFF).astype(np.uint16)
    w16 = w16.reshape(Kt, OUT_F // 16, 32, 2)[..., ::-1].reshape(Kt, OUT_F // 16, 64)
    suh_r = np.ascontiguousarray(suh.reshape(NKB, 128).T)

    in_maps = []
    for c in range(NCORES):
        w16c = w16[:, c * NTS:(c + 1) * NTS, :]
        vs = _pack_vs(w16c)
        svh_r = np.ascontiguousarray(svh[c * NSH:(c + 1) * NSH].reshape(4, 128).T)
        in_maps.append({
            "x_in": x,
            "vs_in": vs,
            "suh_in": suh_r,
            "svh_in": svh_r,
        })

    nc = _get_program()
    res = bass_utils.run_bass_kernel_spmd(nc, in_maps, core_ids=list(range(NCORES)))
    LAST_RESULT = res

    y = np.empty((TOKENS, OUT_F), dtype=np.float32)
    for c in range(NCORES):
        y[:, c * NSH:(c + 1) * NSH] = res.results[c]["y_out"].T
    return y


# revision 11
# speedup vs baseline: 1.2538x; 1.2538x over previous
"""ExllamaV3 trellis-dequant linear layer on 8 Trainium2 NeuronCores.

y = x @ W,  W = diag(suh) . blockH . dequant(trellis) . blockH . diag(svh)

Sharding: column-parallel over out_features (512 cols/core). Per core:
  - trellis words are host-packed (pure layout) into a per-partition uint32
    stream V = (w[j-1]<<16)|w[j] so the on-device dequant is a uniform
    13-op DVE chain (byte extract + exact fp32-split LCG + fp16 bitcast).
  - x is staged k-major (host transpose = layout only); suh and the left
    Hadamard are folded into the dequantized weight via PE matmuls.
  - main matmul runs in fp32r (1-8-11) at full streaming rate.
  - right Hadamard + svh are applied to output tiles via PE transpose +
    matmul; the y shard is returned n-major and assembled on the host.
"""
import os
import numpy as np
from contextlib import ExitStack

import concourse.bass as bass
import concourse.tile as tile
from concourse import bacc, mybir
from concourse import bass_utils

Alu = mybir.AluOpType
f32 = mybir.dt.float32
f32r = mybir.dt.float32r
f16 = mybir.dt.float16
i32 = mybir.dt.int32
u32 = mybir.dt.uint32

TOKENS = 4096
IN_F = 4096
OUT_F = 4096
NCORES = 8
NSH = OUT_F // NCORES          # 512 out cols per core
Kt = IN_F // 16                # 256
NTS = (OUT_F // 16) // NCORES  # 32 trellis tile-cols per core
NKB = IN_F // 128              # 32 contraction chunks
NTT = TOKENS // 128            # 32 token tiles
SPP = NKB * NTS * 2 * 8        # 16384 stream elems per partition

MULT = 89226354
ADD = 64248484
C2h, C2l = MULT >> 16, MULT & 0xFFFF
_C1 = (MULT * 256) & 0xFFFFFFFF
C1h, C1l = _C1 >> 16, _C1 & 0xFFFF
ADDh, ADDl = ADD >> 16, ADD & 0xFFFF

DEQ_FD = 1024                  # dequant batch free-dim (2 k-chunks)
NBATCH = SPP // DEQ_FD         # 16

# ---------------- host-side layout helpers (pure layout, no math) ----------

_p = np.arange(128)
_a_p = _p // 16                 # kt % 8
_r_p = _p % 16                  # k % 16
_c_p = np.where(_r_p < 8, _r_p % 2, 2 + (_r_p % 2))
_tr_p = np.where(_r_p < 8, _r_p // 2, (_r_p - 8) // 2)
SH_H = (20 - 4 * _c_p).astype(np.int32)   # per-partition vh shift
SH_L = (12 - 4 * _c_p).astype(np.int32)   # per-partition vl shift

_s = np.arange(SPP)
_tc_s = _s % 8
_jh_s = (_s // 8) % 2
_nt_s = (_s // 16) % NTS
_kb_s = _s // (16 * NTS)

_KT_IDX = 8 * _kb_s[None, :] + _a_p[:, None]
_J_IDX = 8 * _tc_s[None, :] + 2 * _tr_p[:, None] + _jh_s[None, :]
_JM1_IDX = (_J_IDX - 1) % 64
_NT_IDX = np.broadcast_to(_nt_s[None, :], (128, SPP))


def _hadamard128():
    h = np.array([[1]], dtype=np.int64)
    while h.shape[0] < 128:
        h = np.block([[h, h], [h, -h]])
    return h.astype(np.float32)


def _pack_vs(w16_core):
    """[Kt, NTS, 64] uint16 (pair-swapped) -> [128, SPP] uint32 stream."""
    w0 = w16_core[_KT_IDX, _NT_IDX, _J_IDX].astype(np.uint32)
    w1 = w16_core[_KT_IDX, _NT_IDX, _JM1_IDX].astype(np.uint32)
    return (w1 << 16) | w0


# ---------------- device program ------------------------------------------

def _build_program():
    nc = bacc.Bacc(
        "TRN2",
        target_bir_lowering=False,
        debug=False,
        enable_asserts=False,
        num_devices=NCORES,
    )

    # xT_in: x pre-transposed and tiled on the host:
    #   xT_in[tt, kb, p, t] = x[tt*128 + t, kb*128 + p]
    xT_d = nc.dram_tensor("xT_in", [NTT, NKB, 128, 128], f32, kind="ExternalInput")
    vs_d = nc.dram_tensor("vs_in", [128, SPP], u32, kind="ExternalInput")
    suh_d = nc.dram_tensor("suh_in", [128, NKB], f32, kind="ExternalInput")
    svh_d = nc.dram_tensor("svh_in", [128, 4], f32, kind="ExternalInput")
    y_d = nc.dram_tensor("y_out", [NSH, TOKENS], f32, kind="ExternalOutput")

    had = _hadamard128()
    hL_d = nc.inline_tensor(np.ascontiguousarray(had / 128.0), name="hL")
    hR_d = nc.inline_tensor(np.ascontiguousarray(had), name="hR")
    ident_d = nc.inline_tensor(np.eye(128, dtype=np.float32), name="ident")
    shh_d = nc.inline_tensor(SH_H.reshape(128, 1), name="shh")
    shl_d = nc.inline_tensor(SH_L.reshape(128, 1), name="shl")

    with tile.TileContext(nc) as tc, ExitStack() as ctx:
        cpool = ctx.enter_context(tc.tile_pool(name="consts", bufs=1))
        hL = cpool.tile([128, 128], f32r)
        hR = cpool.tile([128, 128], f32r)
        ident = cpool.tile([128, 128], f32r)
        shh = cpool.tile([128, 1], i32)
        shl = cpool.tile([128, 1], i32)
        suh = cpool.tile([128, NKB], f32)
        svh = cpool.tile([128, 4], f32)
        nc.sync.dma_start(hL[:], hL_d.ap().bitcast(f32r))
        nc.sync.dma_start(hR[:], hR_d.ap().bitcast(f32r))
        nc.sync.dma_start(ident[:], ident_d.ap().bitcast(f32r))
        nc.sync.dma_start(shh[:], shh_d.ap())
        nc.sync.dma_start(shl[:], shl_d.ap())
        nc.sync.dma_start(suh[:], suh_d.ap())
        nc.sync.dma_start(svh[:], svh_d.ap())

        w2pool = ctx.enter_context(tc.tile_pool(name="w2", bufs=NKB))
        W2 = [w2pool.tile([128, NSH], f32r, tag="w2", name=f"w2_{i}")
              for i in range(NKB)]

        vs_pool = ctx.enter_context(tc.tile_pool(name="vsin", bufs=2))
        deq = ctx.enter_context(tc.tile_pool(name="deq", bufs=8))
        wi_pool = ctx.enter_context(tc.tile_pool(name="wi", bufs=2))
        psw_pool = ctx.enter_context(tc.tile_pool(name="psw", bufs=2, space="PSUM"))

        # ---- Phase W: dequant + left Hadamard (x1/128) + suh ----
        for b in range(NBATCH):
            V = vs_pool.tile([128, DEQ_FD], i32)
            nc.sync.dma_start(V[:], vs_d.ap()[:, b * DEQ_FD:(b + 1) * DEQ_FD].bitcast(i32))

            vh = deq.tile([128, DEQ_FD], i32, tag="deq")
            vl = deq.tile([128, DEQ_FD], i32, tag="deq")
            nc.vector.tensor_scalar(vh[:], V[:], shh[:], 0xFF, Alu.logical_shift_right, Alu.bitwise_and)
            nc.vector.tensor_scalar(vl[:], V[:], shl[:], 0xFF, Alu.logical_shift_right, Alu.bitwise_and)

            t1 = deq.tile([128, DEQ_FD], i32, tag="deq")
            nc.vector.tensor_scalar(t1[:], vl[:], C2l, ADDl, Alu.mult, Alu.add)
            slo = deq.tile([128, DEQ_FD], i32, tag="deq")
            nc.vector.scalar_tensor_tensor(slo[:], vh[:], C1l, t1[:], Alu.mult, Alu.add)
            t3 = deq.tile([128, DEQ_FD], i32, tag="deq")
            nc.vector.tensor_scalar(t3[:], vl[:], C2h, ADDh, Alu.mult, Alu.add)
            t4 = deq.tile([128, DEQ_FD], i32, tag="deq")
            nc.vector.scalar_tensor_tensor(t4[:], vh[:], C1h, t3[:], Alu.mult, Alu.add)
            carry = deq.tile([128, DEQ_FD], i32, tag="deq")
            nc.vector.tensor_scalar(carry[:], slo[:], 16, None, Alu.logical_shift_right)
            shi = deq.tile([128, DEQ_FD], i32, tag="deq")
            nc.vector.tensor_tensor(shi[:], t4[:], carry[:], Alu.add)
            rlo = deq.tile([128, DEQ_FD], i32, tag="deq")
            nc.vector.tensor_scalar(rlo[:], slo[:], 0x8FFF, 0x3B60, Alu.bitwise_and, Alu.bitwise_xor)
            rhi = deq.tile([128, DEQ_FD], i32, tag="deq")
            nc.vector.tensor_scalar(rhi[:], shi[:], 0x8FFF, 0x3B60, Alu.bitwise_and, Alu.bitwise_xor)

            wi = wi_pool.tile([128, DEQ_FD], f32r)
            lo16 = rlo[:].bitcast(f16).rearrange("p (f two) -> p f two", two=2)[:, :, 0]
            hi16 = rhi[:].bitcast(f16).rearrange("p (f two) -> p f two", two=2)[:, :, 0]
            nc.vector.tensor_tensor(wi[:], lo16, hi16, Alu.add)

            # left Hadamard (+1/128) per 512-col k-chunk, then suh row scale
            for q in range(DEQ_FD // NSH):
                kb = b * (DEQ_FD // NSH) + q
                psw = psw_pool.tile([128, NSH], f32)
                nc.tensor.matmul(psw[:], hL[:], wi[:, q * NSH:(q + 1) * NSH],
                                 start=True, stop=True)
                nc.scalar.mul(W2[kb][:], psw[:], suh[:, kb:kb + 1])

        # ---- Phase M + Y per token tile ----
        xst_pool = ctx.enter_context(tc.tile_pool(name="xst", bufs=3))
        psy_pool = ctx.enter_context(tc.tile_pool(name="psy", bufs=3, space="PSUM"))
        pst_pool = ctx.enter_context(tc.tile_pool(name="pst", bufs=2, space="PSUM"))
        psz_pool = ctx.enter_context(tc.tile_pool(name="psz", bufs=1, space="PSUM"))
        ysb_pool = ctx.enter_context(tc.tile_pool(name="ysb", bufs=2))
        ytT_pool = ctx.enter_context(tc.tile_pool(name="ytT", bufs=1))
        zsb_pool = ctx.enter_context(tc.tile_pool(name="zsb", bufs=2))

        ytT_group = None
        for tt in range(NTT):
            xst = xst_pool.tile([128, NKB * 128], f32r)
            nc.sync.dma_start(
                xst[:].rearrange("p (kb t) -> p kb t", kb=NKB),
                xT_d.ap()[tt].rearrange("kb p t -> p kb t").bitcast(f32r))

            # main matmul: accumulate y' over all k-chunks
            psy = psy_pool.tile([128, NSH], f32)
            for kb in range(NKB):
                nc.tensor.matmul(psy[:], xst[:, kb * 128:(kb + 1) * 128],
                                 W2[kb][:],
                                 start=(kb == 0), stop=(kb == NKB - 1))

            # y side: transpose y' tiles into n-major staging
            ysb = ysb_pool.tile([128, NSH], f32r)
            nc.scalar.copy(ysb[:], psy[:])
            if tt % 4 == 0:
                ytT_group = ytT_pool.tile([128, 4 * 512], f32r, tag="ytT")
            pst = pst_pool.tile([128, 512], f32r)
            for nb in range(4):
                nc.tensor.transpose(pst[:, nb * 128:(nb + 1) * 128],
                                    ysb[:, nb * 128:(nb + 1) * 128], ident[:])
            dst = ytT_group[:].rearrange("p (nb f) -> p nb f", nb=4)[:, :, (tt % 4) * 128:(tt % 4) * 128 + 128]
            nc.scalar.copy(dst, pst[:])

            if tt % 4 == 3:
                ttg = tt // 4
                for nb in range(4):
                    psz = psz_pool.tile([128, 512], f32)
                    nc.tensor.matmul(psz[:], hR[:],
                                     ytT_group[:, nb * 512:(nb + 1) * 512],
                                     start=True, stop=True)
                    zsb = zsb_pool.tile([128, 512], f32)
                    nc.scalar.mul(zsb[:], psz[:], svh[:, nb:nb + 1])
                    nc.sync.dma_start(
                        y_d.ap()[nb * 128:(nb + 1) * 128, ttg * 512:(ttg + 1) * 512],
                        zsb[:])

    nc.compile()
    return nc


_NC_CACHE = None
LAST_RESULT = None


def _get_program():
    global _NC_CACHE
    if _NC_CACHE is None:
        _NC_CACHE = _build_program()
    return _NC_CACHE


def kernel(x, trellis, suh, svh):
    global LAST_RESULT
    x = np.asarray(x, dtype=np.float32)
    trellis = np.asarray(trellis)
    suh = np.asarray(suh, dtype=np.float32)
    svh = np.asarray(svh, dtype=np.float32)

    # host layout prep (pure re-layout, no arithmetic)
    w16 = (trellis.astype(np.uint32) & 0xFFFF).astype(np.uint16)
    w16 = w16.reshape(Kt, OUT_F // 16, 32, 2)[..., ::-1].reshape(Kt, OUT_F // 16, 64)
    suh_r = np.ascontiguousarray(suh.reshape(NKB, 128).T)
    # xT[tt, kb, p, t] = x[tt*128+t, kb*128+p]
    xT = np.ascontiguousarray(
        x.reshape(NTT, 128, NKB, 128).transpose(0, 2, 3, 1))

    in_maps = []
    for c in range(NCORES):
        w16c = w16[:, c * NTS:(c + 1) * NTS, :]
        vs = _pack_vs(w16c)
        svh_r = np.ascontiguousarray(svh[c * NSH:(c + 1) * NSH].reshape(4, 128).T)
        in_maps.append({
            "xT_in": xT,
            "vs_in": vs,
            "suh_in": suh_r,
            "svh_in": svh_r,
        })

    nc = _get_program()
    res = bass_utils.run_bass_kernel_spmd(nc, in_maps, core_ids=list(range(NCORES)))
    LAST_RESULT = res

    y = np.empty((TOKENS, OUT_F), dtype=np.float32)
    for c in range(NCORES):
        y[:, c * NSH:(c + 1) * NSH] = res.results[c]["y_out"].T
    return y
